# revision 2
# baseline (speedup 1.0000x reference)
"""Trainium2 Bass kernel for nn_Attention_61907658605177.

Self-attention where q == k == v, each equal to the input x reinterpreted as
[H=16, B=2, S=2048, hd=64].  Output is att.swapaxes(1,2).reshape(-1, 1024).

Sharding: the 32 independent (h, b) pairs are split 4-per-core across the
8 NeuronCores (pure data parallelism, no collectives).  Each pair's slice of
x's flat buffer is contiguous, so core i receives x.reshape(32, 2048, 64)[4i:4i+4].

Per-core algorithm (per head, S=2048, hd=64).  The score matrix is SYMMETRIC
(q == k), so only the upper-triangle tiles are computed and exp'd — 136 of 256
[128,128] tiles — cutting the ACT-engine exp work (the kernel bottleneck)
nearly in half versus exp'ing the full matrix:

  - mm1 computes upper row segments scores[t, t*128:2048] on TensorE in fp16,
    split into <=512-column PSUM-bank pieces that alternate between the two
    K=64 PE row-groups (qT duplicated across both partition halves) so
    adjacent pieces run concurrently.  Two PSUM ring slots (R0: rows 0-7,
    R1: rows 8-15) host one row each; rows are processed interleaved
    (0,8,1,9,...) so each slot's exp covers the other slot's mm1 refill.
  - ACT exps each row segment straight out of PSUM into a packed fp16 ebuf:
    E = exp(scores/8 - 8); the -8 cancels in softmax.
  - The strictly-lower tiles of E are NOT exp'd: one DMA-engine XBAR
    transpose per row turns the stored segment row t into the stacked tiles
    E[c,t] (c > t) — by symmetry exp(S)^T tiles.  This runs on the otherwise
    idle DMA engines, costing no ACT/PE/DVE time.
  - mm2: att[t] accumulates over chunks c in PSUM; the operand tile for
    (t, c<=t) is the stored upper tile read directly (symmetry again), and
    for c>t the DMA-transposed tile.  v carries a ones column so column 64 of
    the accumulator is the softmax denominator.
  - VectorE computes 1/denominator and scales while evacuating PSUM; DMA out.
"""

import sys

if "/opt/trn_rl_repo" not in sys.path:
    sys.path.insert(0, "/opt/trn_rl_repo")

import numpy as np

import concourse.bass as bass
import concourse.mybir as mybir
import concourse.tile as tile
from concourse.bass_utils import run_bass_kernel_spmd
from concourse.masks import make_identity

F32 = mybir.dt.float32
F16 = mybir.dt.float16

H, B, S, E_DIM, HD = 16, 2, 2048, 1024, 64
N_CORES = 8
PAIRS = 4            # (h, b) heads per core
T = S // 128         # 16 q-tiles per head
SCALE = 0.125        # 1/sqrt(DK) with DK=64
EXP_BIAS = -8.0      # constant shift inside exp; cancels in softmax

# upper-triangle row geometry (elements are fp16 in ebuf, fp32 in PSUM)
W = [(T - t) * 128 for t in range(T)]        # row segment widths
OFF = [0] * T                                 # ebuf offset of row t
for t in range(1, T):
    OFF[t] = OFF[t - 1] + W[t - 1]
EU = OFF[-1] + W[-1]                          # 17408 elems/partition
NT = [T - 1 - t for t in range(T)]            # transposed tiles per row
ETO = [0] * T                                 # eT tile offset of row t
for t in range(1, T):
    ETO[t] = ETO[t - 1] + NT[t - 1]
NET = ETO[-1] + NT[-1]                        # 120 tiles

# rows 0-7 live in the 4-bank slot R0, rows 8-15 in the 2-bank slot R1;
# interleaved order so each exp covers the other slot's mm1 refill
SEQ = []
for t in range(8):
    SEQ += [t, t + 8]

ts = bass.ts


def _apply_tile_drain_patch():
    """walrus in this toolchain rejects instructions carrying too many sync
    commands; re-emit the kernel-tail drain's waits as standalone ops."""
    from bass_rust import ScopedClock

    def _drain_and_barrier_split(self, tick_clock, wait_clock):
        nc = self.nc
        drain = nc.sync.drain()
        wait_clock.add_sem_waits(
            drain.ins, ScopedClock({None: tick_clock.global_clock})
        )
        si = drain.ins.sync_info
        waits = list(si.on_wait or []) if si is not None else []
        if len(waits) > 1:
            si.on_wait = []
            name_to_handle = {h.name: h for h in self.sems.allocated().values()}
            for w in waits:
                nc.sync.wait_ge(name_to_handle[w.ant_name], w.wait_value)
            nc.sync.drain()
        nc.all_engine_barrier()
        popped = nc._tile_sem_poison_stack.pop()
        assert popped is self._sem_poison
        nc.clear_and_free_semaphores(list(self.sems.allocated().values()))
        nc.all_engine_barrier()

    tile.TileContext._drain_and_barrier = _drain_and_barrier_split


_DMA_LIKE = (
    mybir.InstDMACopy,
    mybir.InstDmaTransposeAnt,
    mybir.InstDMA,
    mybir.InstCollectiveCompute,
    mybir.InstDrain,
    mybir.InstNoOp,
)


def _split_sync_waits(nc):
    """walrus rejects instructions with more than ~2 total sync commands
    (waits + updates; DMA pseudos tolerate only 1 wait).  Move overflow waits
    onto preceding same-engine NoOps, one wait each."""
    nsplit = 0
    for f in nc.m.functions:
        for b in f.blocks:
            live = b.instructions
            insts = list(live)
            out_insts = []
            changed = False
            for inst in insts:
                si = getattr(inst, "sync_info", None)
                waits = list(si.on_wait) if (si is not None and si.on_wait) else []
                nupd = len(si.on_update) if (si is not None and si.on_update) else 0
                maxw = max(0, 2 - nupd)
                if isinstance(inst, _DMA_LIKE):
                    maxw = min(maxw, 1)
                if len(waits) > maxw:
                    si.on_wait = waits[-maxw:] if maxw > 0 else []
                    for w in (waits[:-maxw] if maxw > 0 else waits):
                        nsplit += 1
                        nop = mybir.InstNoOp(
                            name=f"wsplit-{nsplit}-{inst.name}",
                            engine=inst.engine,
                            sync_info=mybir.SyncInfo(on_wait=[w], on_update=[]),
                        )
                        out_insts.append(nop)
                    changed = True
                out_insts.append(inst)
            if changed:
                live[:] = out_insts
    return nsplit


def build_kernel(loop_reps: int = 1):
    _apply_tile_drain_patch()
    nc = bass.Bass()
    x = nc.declare_dram_parameter("x", [PAIRS, S, HD], F32, isOutput=False)
    out = nc.declare_dram_parameter("out", [PAIRS, S, HD], F32, isOutput=True)

    with tile.TileContext(nc) as tc:
        with (
            tc.tile_pool(name="singles", bufs=1) as singles,
            tc.tile_pool(name="qn", bufs=4) as qn_pool,
            tc.tile_pool(name="v", bufs=3) as v_pool,
            tc.tile_pool(name="qT2", bufs=2) as qT2_pool,
            tc.tile_pool(name="ebuf", bufs=2) as e_pool,
            tc.tile_pool(name="eT", bufs=2) as eT_pool,
            tc.tile_pool(name="sums", bufs=2) as sums_pool,
            tc.tile_pool(name="outsb", bufs=2) as out_pool,
            tc.tile_pool(name="r0", bufs=1, space="PSUM") as r0_pool,
            tc.tile_pool(name="r1", bufs=1, space="PSUM") as r1_pool,
            tc.tile_pool(name="attp0", bufs=1, space="PSUM") as att0_pool,
            tc.tile_pool(name="attp1", bufs=1, space="PSUM") as att1_pool,
        ):
            ident = singles.tile([128, 128], F16)
            make_identity(nc, ident)
            bias_tile = singles.tile([128, 1], F32)
            nc.vector.memset(bias_tile, EXP_BIAS)
            # dummy exp: pulls the ACT exp table load (~2.7us) off the
            # critical path by overlapping it with the first input DMA
            warm = singles.tile([128, 1], F32)
            nc.scalar.activation(
                warm[:], bias_tile[:], mybir.ActivationFunctionType.Exp
            )

            # PSUM map (8 banks): R0 4, R1 2, att0 1, att1 1.
            # Slot tensors are separate so Tile's tensor-granular WAR tracking
            # lets one slot's mm1 refill proceed while the other's exp runs.
            R0 = r0_pool.tile([128, 2048], F32, name="scoresR0")
            R1 = r1_pool.tile([128, 1024], F32, name="scoresR1")
            att01 = (
                att0_pool.tile([128, HD + 1], F32, name="att0"),
                att1_pool.tile([128, HD + 1], F32, name="att1"),
            )
            # [64, 2048] f16 staging view of R1 for the input transposes
            qtp_view = R1[0:64, :].bitcast(F16)

            st = [dict() for _ in range(PAIRS)]

            def alloc_qn(p):
                st[p]["qn"] = qn_pool.tile([128, T, HD], F32, tag="qn", name="qn")

            def dma_in(p):
                # partition pp holds input rows {16*pp + j}; q-tile t is the
                # rows {16*pp + t}.  Attention is row-order agnostic (the
                # permutation applies to q and k alike, preserving symmetry),
                # and the output DMA maps back to natural order.
                xr = x[p].rearrange("(pp j) d -> pp j d", j=16)
                nc.sync.dma_start(out=st[p]["qn"][:], in_=xr)

            def cast(p):
                v = v_pool.tile([128, T, HD + 1], F16, tag="v", name="v")
                nc.vector.memset(v[:, :, HD : HD + 1], 1.0)
                st[p]["v"] = v
                nc.vector.tensor_copy(v[:, :, 0:HD], st[p]["qn"][:])
                st[p]["qT2"] = qT2_pool.tile([128, S], F16, tag="qT2", name="qT2")
                st[p]["recips"] = sums_pool.tile(
                    [128, T], F32, tag="recips", name="recips"
                )
                st[p]["outsb"] = out_pool.tile(
                    [128, T, HD], F32, tag="outsb", name="outsb"
                )

            def tchunk(p, k):
                # four [128,64] -> [64,128] PE transposes staged in R1 (f16
                # view), then copied into qT2 and duplicated into the upper
                # partition half for the K=64 row-group concurrency trick
                v = st[p]["v"]
                qT2 = st[p]["qT2"]
                for i, j in enumerate(range(4 * k, 4 * k + 4)):
                    nc.tensor.transpose(
                        qtp_view[:, ts(i, 128)], v[:, j, 0:HD], ident[:]
                    )
                nc.vector.tensor_copy(qT2[0:64, ts(k, 512)], qtp_view[:, 0:512])
                nc.vector.tensor_copy(
                    qT2[64:128, ts(k, 512)], qT2[0:64, ts(k, 512)]
                )

            def alloc_e(p):
                st[p]["ebuf"] = e_pool.tile([128, EU], F16, tag="ebuf", name="ebuf")
                st[p]["eT"] = eT_pool.tile(
                    [128, NET, 128], F16, tag="eT", name="eT"
                )

            def mm1(p, r):
                # upper row segment scores[r, r*128:2048] in <=512-col pieces,
                # alternating PE row-groups so adjacent pieces run concurrently
                qT2 = st[p]["qT2"]
                slot = R0 if r < 8 else R1
                w = W[r]
                o, k = 0, 0
                while o < w:
                    pw = min(512, w - o)
                    rows = slice(0, 64) if k % 2 == 0 else slice(64, 128)
                    nc.tensor.matmul(
                        slot[:, o : o + pw],
                        qT2[rows, ts(r, 128)],
                        qT2[rows, r * 128 + o : r * 128 + o + pw],
                    )
                    o += pw
                    k += 1

            def exp_row(p, r):
                slot = R0 if r < 8 else R1
                nc.scalar.activation(
                    st[p]["ebuf"][:, OFF[r] : OFF[r] + W[r]],
                    slot[:, 0 : W[r]],
                    mybir.ActivationFunctionType.Exp,
                    bias=bias_tile[:],
                    scale=SCALE,
                )

            def dmaT(p, r):
                # XBAR transpose of the strictly-upper part of row r:
                # eT[:, ETO[r]+j, :] = E_tiles[r+1+j, r] for j < NT[r]
                if NT[r] == 0:
                    return
                ebuf, eT = st[p]["ebuf"], st[p]["eT"]
                nc.sync.dma_start_transpose(
                    out=eT[:, ETO[r] : ETO[r] + NT[r], :],
                    in_=ebuf[:, OFF[r] + 128 : OFF[r] + W[r]],
                )

            def mm2norm(p, t):
                ebuf, eT, v = st[p]["ebuf"], st[p]["eT"], st[p]["v"]
                slot = att01[t % 2][:]
                for c in range(T):
                    if c <= t:
                        lhsT = ebuf[:, OFF[c] + (t - c) * 128 : OFF[c] + (t - c) * 128 + 128]
                    else:
                        lhsT = eT[:, ETO[t] + (c - t - 1), :]
                    nc.tensor.matmul(
                        slot,
                        lhsT,
                        v[:, c, :],
                        start=(c == 0),
                        stop=(c == T - 1),
                    )
                rc = st[p]["recips"]
                nc.vector.reciprocal(rc[:, t : t + 1], slot[:, HD : HD + 1])
                nc.vector.tensor_scalar_mul(
                    st[p]["outsb"][:, t, :], slot[:, 0:HD], rc[:, t : t + 1]
                )

            def dma_out(p, quarter=None):
                odram = out[p].rearrange("(pp t) d -> pp t d", t=16)
                if quarter is None:
                    nc.sync.dma_start(out=odram, in_=st[p]["outsb"][:])
                else:
                    q4 = ts(quarter, 4)
                    nc.sync.dma_start(
                        out=odram[:, q4, :], in_=st[p]["outsb"][:, q4, :]
                    )

            def prologue0():
                # head-0 SBUF prep; qn(0)/qn(1) were DMA'd pre-loop (iter 1)
                # or during the previous iteration's phase 3 (steady state)
                cast(0)
                for k in range(4):
                    tchunk(0, k)
                alloc_e(0)

            def aux(p, i):
                # non-critical work scheduled into step i of head p's phase
                if p >= 1:
                    mm2norm(p - 1, i)
                    if i == 15:
                        dma_out(p - 1)
                if p < PAIRS - 1:
                    # prologue for head p+1
                    if i == 2:
                        cast(p + 1)
                    elif i in (4, 6, 8, 10):
                        tchunk(p + 1, (i - 4) // 2)
                        if i == 10:
                            alloc_e(p + 1)
                    elif i == 13:
                        # input DMA for head p+2, 1.5 phases ahead of its cast
                        if p + 2 < PAIRS:
                            alloc_qn(p + 2)
                            dma_in(p + 2)
                else:
                    # head 3's phase: its own mm2 tiles 0..6 unlock as rows
                    # complete (tile t needs rows 0..t exp'd + its dmaT)
                    if i >= 2 and i % 2 == 0 and (i - 2) // 2 <= 6:
                        mm2norm(p, (i - 2) // 2)
                    if i == 0:
                        # next iteration's head-0/1 input DMAs; lands during
                        # this phase so the next prologue never stalls
                        alloc_qn(0)
                        dma_in(0)
                    elif i == 8:
                        alloc_qn(1)
                        dma_in(1)

            def emit_body():
                prologue0()
                for p in range(PAIRS):
                    for i in range(16):
                        mm1(p, SEQ[i])
                        if i >= 1:
                            exp_row(p, SEQ[i - 1])
                            dmaT(p, SEQ[i - 1])
                        aux(p, i)
                    exp_row(p, SEQ[15])
                    dmaT(p, SEQ[15])
                # tail: head 3 tiles 7..15 (tile 7+ needs row 7, exp'd at the
                # phase's penultimate step)
                p = PAIRS - 1
                mm2norm(p, 7)
                dma_out(p, quarter=0)
                dma_out(p, quarter=1)
                for t in range(8, T):
                    mm2norm(p, t)
                    if t in (11, 15):
                        dma_out(p, quarter=(t - 8) // 4 + 2)

            # pre-loop peel: head 0/1 input DMAs + PE/ACT warm-up
            alloc_qn(0)
            dma_in(0)
            alloc_qn(1)
            dma_in(1)
            # HAM warm-up: dummy matmuls while the DMA is in flight, so the
            # PE array is un-throttled when the transposes and mm1 arrive
            for _ in range(40):
                nc.tensor.matmul(att01[0][:], ident[:], ident[:, 0 : HD + 1])

            if loop_reps > 1:
                with tc.For_i(
                    0, loop_reps, 1, hint_engines=(mybir.EngineType.PE,)
                ):
                    emit_body()
            else:
                emit_body()

    _split_sync_waits(nc)
    return nc


_NC_CACHE = None


def kernel(x: np.ndarray) -> np.ndarray:
    global _NC_CACHE
    if _NC_CACHE is None:
        _NC_CACHE = build_kernel()
    nc = _NC_CACHE

    x = np.asarray(x, dtype=np.float32)
    xr = np.reshape(x, (H * B, S, HD))  # flat-buffer reinterpret: pair = h*B + b
    in_maps = [
        {"x": np.ascontiguousarray(xr[i * PAIRS : (i + 1) * PAIRS])}
        for i in range(N_CORES)
    ]
    res = run_bass_kernel_spmd(nc, in_maps, core_ids=list(range(N_CORES)))
    att = np.concatenate([res.results[i]["out"] for i in range(N_CORES)], axis=0)
    att = att.reshape(H, B, S, HD).swapaxes(1, 2).reshape(-1, E_DIM)
    return np.ascontiguousarray(att.astype(np.float32))
